# revision 13
# baseline (speedup 1.0000x reference)
"""Multi-head causal attention (B=4, L=2048, D=1024, H=16) on 8 TRN2 NeuronCores.

Sharding: core c handles batch b = c//2 and head-group hg = c%2 (8 heads, 512 dims).
Each core computes Q/K/V projections for its heads, causal attention, and a
partial output projection (its 512 input dims of Wo). Host sums the two
partials per batch.

Matmuls run in fp16 (same PE rate as bf16, better mantissa; ~1e-3 rel err).
Structure keeps the PE busy through the attention phase (Q-projection of the
next q-chunk and output-projection of the previous q-chunk are interleaved
with S/AV matmuls) so the HAM activity monitor holds the 2.4 GHz clock.
"""
import sys

sys.path.insert(0, "/opt/trn_rl_repo")

import numpy as np

import concourse.bass as bass
import concourse.mybir as mybir
import concourse.tile as tile
from concourse import bacc
from concourse.masks import make_identity

F32 = mybir.dt.float32
F16 = mybir.dt.float16
MM = F16
AF = mybir.ActivationFunctionType

B, L, D, H = 4, 2048, 1024, 16
DK = 64          # head dim
E = 512          # per-core head dims (8 heads)
NL = L // 128    # 16 l-tiles
ND = D // 128    # 8 d-tiles (contraction for projections)
NE = E // 128    # 4 e-tiles
NJ = L // 512    # 4 q-chunks
NK = L // 128    # 16 k-tiles
NDO = E // 128   # 4 d-tiles for out-proj contraction

PT_BUFS = 2

_CACHE = {}


def build_program():
    nc = bacc.Bacc("TRN2", target_bir_lowering=False, debug=False, num_devices=8)

    xb = nc.dram_tensor("xb", [L, D], F32, kind="ExternalInput")
    wq = nc.dram_tensor("wq", [E, D], F32, kind="ExternalInput")
    wk = nc.dram_tensor("wk", [E, D], F32, kind="ExternalInput")
    wv = nc.dram_tensor("wv", [E, D], F32, kind="ExternalInput")
    wo = nc.dram_tensor("wo", [D, E], F32, kind="ExternalInput")
    masks = nc.dram_tensor("masks", [4, 128, 512], F16, kind="ExternalInput")
    out = nc.dram_tensor("out", [L, D], F32, kind="ExternalOutput")

    with tile.TileContext(nc) as tc:
        with (
            tc.tile_pool(name="const", bufs=1) as constp,
            tc.tile_pool(name="big", bufs=1) as bigp,
            tc.tile_pool(name="stripp", bufs=2) as stripp,
            tc.tile_pool(name="qtc", bufs=2) as qtcp,
            tc.tile_pool(name="ptp", bufs=PT_BUFS) as ptp,
            tc.tile_pool(name="smallp", bufs=2) as smallp,
            tc.tile_pool(name="psX", bufs=1, space="PSUM") as psX,
            tc.tile_pool(name="psP", bufs=2, space="PSUM") as psP,
            tc.tile_pool(name="psS", bufs=3, space="PSUM") as psS,
            tc.tile_pool(name="psAV", bufs=2, space="PSUM") as psAV,
        ):
            ident = constp.tile([128, 128], F32)
            make_identity(nc, ident[:])
            ident_h = constp.tile([128, 128], F16)
            make_identity(nc, ident_h[:])
            ones_f = constp.tile([128, 16], F32)
            nc.vector.memset(ones_f[:], 1.0)
            ones_r = constp.tile([1, 64], MM)
            nc.vector.tensor_copy(ones_r[:], ones_f[0:1, 0:1].to_broadcast((1, 64)))
            masks_sb = constp.tile([128, 4, 512], F16)
            for m in range(4):
                nc.sync.dma_start(masks_sb[:, m, :], masks[m])

            xT = bigp.tile([128, ND, L], MM)          # x^T, [d-in-tile, d-tile, l]
            WTq = bigp.tile([128, ND, E], MM)
            WTk = bigp.tile([128, ND, E], MM)
            WTv = bigp.tile([128, ND, E], MM)
            KT = bigp.tile([128, NE, L], MM)
            VT = bigp.tile([128, NE, L], MM)
            attT = bigp.tile([128, NDO, L], MM)
            WoT = bigp.tile([128, NDO, D], MM)
            Vaug = bigp.tile([128, 8, NK, 65], MM)    # per-head V [k, dk] + ones col

            # ---- load (cast to fp16 via SWDGE) + xbar dma-transpose ----
            for wdram, WT in ((wq, WTq), (wk, WTk), (wv, WTv)):
                for et in range(NE):
                    strip = stripp.tile([128, D], F16, tag="strip")
                    nc.gpsimd.dma_start(strip[:], wdram[et * 128:(et + 1) * 128, :])
                    nc.sync.dma_start(
                        WT[:, :, et * 128:(et + 1) * 128], strip[:], transpose=True
                    )
            for et8 in range(ND):
                strip = stripp.tile([128, E], F16, tag="stripo", name="stripo")
                nc.gpsimd.dma_start(strip[:], wo[et8 * 128:(et8 + 1) * 128, :])
                nc.sync.dma_start(
                    WoT[:, :, et8 * 128:(et8 + 1) * 128], strip[:], transpose=True
                )

            # ---- load + transpose x ----
            for lt in range(NL):
                strip = stripp.tile([128, D], F16, tag="strip")
                nc.gpsimd.dma_start(strip[:], xb[lt * 128:(lt + 1) * 128, :])
                nc.sync.dma_start(
                    xT[:, :, lt * 128:(lt + 1) * 128], strip[:], transpose=True
                )

            # ones column of Vaug (written once; V data fills in chunk-wise)
            nc.vector.tensor_copy(
                Vaug[:, :, :, 64:65],
                ones_f[:, 0:1].to_broadcast((128, 8, NK, 1)),
            )

            def kv_proj_group(WT, OUT, et, jc):
                pp = psP.tile([128, 512], F32, tag="pp", name="pp")
                for dt in range(ND):
                    nc.tensor.matmul(
                        pp[:],
                        WT[:, dt, et * 128:(et + 1) * 128],
                        xT[:, dt, jc * 512:(jc + 1) * 512],
                        start=(dt == 0),
                        stop=(dt == ND - 1),
                    )
                nc.vector.tensor_copy(OUT[:, et, jc * 512:(jc + 1) * 512], pp[:])
                if OUT is VT:
                    # V^T -> V natural for the 2 heads in this e-tile's rows,
                    # this l-chunk's 4 k-tiles. The xbar transpose needs a
                    # 256B-aligned output, so stage padded then DVE-copy.
                    for par in range(2):
                        h = et * 2 + par
                        hp = par * 64
                        vstg = stripp.tile([128, 4, 128], F16, tag="vstg", name="vstg")
                        for kk in range(4):
                            kt = 4 * jc + kk
                            nc.sync.dma_start(
                                vstg[:, kk, 0:64],
                                VT[hp:hp + 64, et, kt * 128:(kt + 1) * 128],
                                transpose=True,
                            )
                        nc.vector.tensor_copy(
                            Vaug[:, h, 4 * jc:4 * jc + 4, 0:64], vstg[:, :, 0:64]
                        )

            def q_proj_group(j, qtile, et):
                pp = psP.tile([128, 512], F32, tag="pp", name="pp")
                for dt in range(ND):
                    nc.tensor.matmul(
                        pp[:],
                        WTq[:, dt, et * 128:(et + 1) * 128],
                        xT[:, dt, j * 512:(j + 1) * 512],
                        start=(dt == 0),
                        stop=(dt == ND - 1),
                    )
                nc.vector.tensor_copy(qtile[:, et, :], pp[:])

            def out_proj_group(lt, ec):
                op = psP.tile([128, 512], F32, tag="pp", name="op")
                for dt in range(NDO):
                    nc.tensor.matmul(
                        op[:],
                        attT[:, dt, lt * 128:(lt + 1) * 128],
                        WoT[:, dt, ec * 512:(ec + 1) * 512],
                        start=(dt == 0),
                        stop=(dt == NDO - 1),
                    )
                ot = smallp.tile([128, 512], F32, tag="ot", name="ot")
                nc.vector.tensor_copy(ot[:], op[:])
                nc.sync.dma_start(
                    out[lt * 128:(lt + 1) * 128, ec * 512:(ec + 1) * 512],
                    ot[:],
                )

            # ---- attention, q-chunk outer / head inner, with PE filler ----
            # upfront: projections for chunk 0
            for et in range(NE):
                kv_proj_group(WTk, KT, et, 0)
                kv_proj_group(WTv, VT, et, 0)
            qtiles = {}
            qtiles[0] = qtcp.tile([128, NE, 512], MM, tag="qt", name="qt0")
            for et in range(NE):
                q_proj_group(0, qtiles[0], et)
            for j in range(NJ):
                filler = []
                if j + 1 < NJ:
                    qtiles[j + 1] = qtcp.tile([128, NE, 512], MM, tag="qt", name=f"qt{j+1}")
                    for et in range(NE):
                        filler.append(lambda et=et, jn=j + 1: kv_proj_group(WTk, KT, et, jn))
                        filler.append(lambda et=et, jn=j + 1: kv_proj_group(WTv, VT, et, jn))
                        filler.append(lambda et=et, jn=j + 1, qt=qtiles[j + 1]: q_proj_group(jn, qt, et))
                if j >= 1:
                    for lt in range(4 * (j - 1), 4 * (j - 1) + 4):
                        for ec in range(2):
                            filler.append(lambda lt=lt, ec=ec: out_proj_group(lt, ec))
                nfil = len(filler)
                nkt = 4 * (j + 1)
                QTc = qtiles[j]
                for h in range(8):
                    for f in filler[nfil * h // 8: nfil * (h + 1) // 8]:
                        f()
                    hp = (h % 2) * 64
                    hb = h // 2
                    PT = ptp.tile([128, NK, 512], MM, tag="pt")
                    for kt in range(nkt):
                        s_ps = psS.tile([128, 512], F32, tag="s")
                        nc.tensor.matmul(
                            s_ps[:],
                            KT[hp:hp + 64, hb, kt * 128:(kt + 1) * 128],
                            QTc[hp:hp + 64, hb, :],
                        )
                        nc.scalar.activation(
                            PT[:, kt, :], s_ps[:], AF.Exp, scale=0.125
                        )
                        if kt >= nkt - 4:
                            nc.vector.tensor_mul(
                                PT[:, kt, :],
                                PT[:, kt, :],
                                masks_sb[:, kt - (nkt - 4), :],
                            )
                    att_ps = psAV.tile([65, 512], F32, tag="av")
                    for kt in range(nkt):
                        nc.tensor.matmul(
                            att_ps[:],
                            Vaug[:, h, kt, 0:65],
                            PT[:, kt, :],
                            start=(kt == 0),
                            stop=(kt == nkt - 1),
                        )
                    recip = smallp.tile([1, 512], MM, tag="recip")
                    with nc.allow_low_precision(reason="fp16 rounding for PE broadcast"):
                        nc.vector.reciprocal(recip[:], att_ps[64:65, :])
                    bc_ps = psX.tile([64, 512], F32, tag="x", name="bc_ps")
                    nc.tensor.matmul(bc_ps[:], ones_r[:], recip[:])
                    bc_sb = smallp.tile([64, 512], F32, tag="bcsb")
                    nc.vector.tensor_copy(bc_sb[:], bc_ps[:])
                    nc.vector.tensor_mul(
                        attT[hp:hp + 64, hb, j * 512:(j + 1) * 512],
                        att_ps[0:64, :],
                        bc_sb[:],
                    )
            for lt in range(4 * (NJ - 1), 4 * (NJ - 1) + 4):
                for ec in range(2):
                    out_proj_group(lt, ec)

    nc.compile()
    return nc


def build_masks():
    kp = np.arange(128)[:, None]
    qf = np.arange(512)[None, :]
    return np.stack(
        [(qf >= kp + 128 * m).astype(np.float16) for m in range(4)]
    )


def _get_program():
    if "nc" not in _CACHE:
        _CACHE["nc"] = build_program()
    return _CACHE["nc"]


def make_in_maps(x, Wq, Wk, Wv, Wo):
    x = np.asarray(x, dtype=np.float32)
    Wq = np.asarray(Wq, dtype=np.float32)
    Wk = np.asarray(Wk, dtype=np.float32)
    Wv = np.asarray(Wv, dtype=np.float32)
    Wo = np.asarray(Wo, dtype=np.float32)
    masks = build_masks()
    in_maps = []
    for c in range(8):
        b, hg = c // 2, c % 2
        sl = slice(hg * E, (hg + 1) * E)
        in_maps.append(
            {
                "xb": np.ascontiguousarray(x[b]),
                "wq": np.ascontiguousarray(Wq[sl]),
                "wk": np.ascontiguousarray(Wk[sl]),
                "wv": np.ascontiguousarray(Wv[sl]),
                "wo": np.ascontiguousarray(Wo[:, sl]),
                "masks": masks,
            }
        )
    return in_maps


def kernel(x, Wq, Wk, Wv, Wo, **run_kwargs):
    from concourse import bass_utils

    nc = _get_program()
    in_maps = make_in_maps(x, Wq, Wk, Wv, Wo)
    res = bass_utils.run_bass_kernel_spmd(
        nc, in_maps, core_ids=list(range(8)), **run_kwargs
    )
    out = np.empty((B, L, D), np.float32)
    for b in range(B):
        out[b] = res.results[2 * b]["out"] + res.results[2 * b + 1]["out"]
    _CACHE["last_result"] = res
    return out


# revision 14
# speedup vs baseline: 1.1101x; 1.1101x over previous
"""Multi-head causal attention (B=4, L=2048, D=1024, H=16) on 8 TRN2 NeuronCores.

Sharding: core c handles batch b = c//2 and head-group hg = c%2 (8 heads, 512
dims). Each core computes Q/K/V projections for its heads, causal attention,
and a partial output projection (its 512 input dims of Wo). Host sums the two
partials per batch.

fp16 matmuls. The PE instruction stream is kept dense through the attention
phase by interleaving projection work (K/V/Q of the next q-chunk, output
projection of the previous q-chunk) between attention matmuls at fine grain —
otherwise the activity monitor drops the PE clock from 2.4 to 1.2 GHz.
"""
import sys

sys.path.insert(0, "/opt/trn_rl_repo")

import numpy as np

import concourse.bass as bass
import concourse.mybir as mybir
import concourse.tile as tile
from concourse import bacc
from concourse.masks import make_identity

F32 = mybir.dt.float32
F16 = mybir.dt.float16
MM = F16
AF = mybir.ActivationFunctionType

B, L, D, H = 4, 2048, 1024, 16
DK = 64
E = 512
NL = L // 128
ND = D // 128
NE = E // 128
NJ = L // 512
NK = L // 128
NDO = E // 128

PT_BUFS = 2

_CACHE = {}


def build_program():
    nc = bacc.Bacc("TRN2", target_bir_lowering=False, debug=False, num_devices=8)

    xb = nc.dram_tensor("xb", [L, D], F32, kind="ExternalInput")
    wq = nc.dram_tensor("wq", [E, D], F32, kind="ExternalInput")
    wk = nc.dram_tensor("wk", [E, D], F32, kind="ExternalInput")
    wv = nc.dram_tensor("wv", [E, D], F32, kind="ExternalInput")
    wo = nc.dram_tensor("wo", [D, E], F32, kind="ExternalInput")
    masks = nc.dram_tensor("masks", [4, 128, 512], F16, kind="ExternalInput")
    out = nc.dram_tensor("out", [L, D], F32, kind="ExternalOutput")

    with tile.TileContext(nc) as tc:
        with (
            tc.tile_pool(name="const", bufs=1) as constp,
            tc.tile_pool(name="big", bufs=1) as bigp,
            tc.tile_pool(name="stripp", bufs=3) as stripp,
            tc.tile_pool(name="qtc", bufs=2) as qtcp,
            tc.tile_pool(name="ptp", bufs=PT_BUFS) as ptp,
            tc.tile_pool(name="smallp", bufs=2) as smallp,
            tc.tile_pool(name="psX", bufs=1, space="PSUM") as psX,
            tc.tile_pool(name="psP", bufs=2, space="PSUM") as psP,
            tc.tile_pool(name="psS", bufs=3, space="PSUM") as psS,
            tc.tile_pool(name="psAV", bufs=2, space="PSUM") as psAV,
        ):
            ones_f = constp.tile([128, 16], F32)
            nc.vector.memset(ones_f[:], 1.0)
            ones_r = constp.tile([1, 64], MM)
            nc.vector.tensor_copy(ones_r[:], ones_f[0:1, 0:1].to_broadcast((1, 64)))
            masks_sb = constp.tile([128, 4, 512], F16)
            for m in range(4):
                nc.sync.dma_start(masks_sb[:, m, :], masks[m])

            xT = bigp.tile([128, ND, L], MM)       # x^T  [d-in-tile, d-tile, l]
            WTq = bigp.tile([128, ND, E], MM)      # Wq^T [d-in-tile, d-tile, e]
            WTk = bigp.tile([128, ND, E], MM)
            WTv = bigp.tile([128, ND, E], MM)
            KT = bigp.tile([128, NE, L], MM)       # K^T  [dk (2 heads), e-tile, k]
            attT = bigp.tile([128, NDO, L], MM)
            WoT = bigp.tile([128, NDO, D], MM)
            Vaug = bigp.tile([128, NK, 8, 65], MM)  # V natural per (k-tile, head) + ones

            nc.vector.tensor_copy(
                Vaug[:, :, :, 64:65],
                ones_f[:, 0:1].to_broadcast((128, NK, 8, 1)),
            )

            # ---------- emission helpers ----------
            def load_w(wdram, WT):
                # [e-rows, d] fp32 -> fp16 strip -> xbar transpose to [d, e-slice]
                for et in range(WT.shape[2] // 128):
                    strip = stripp.tile([128, D], F16, tag="strip", name="strip")
                    nc.gpsimd.dma_start(strip[:], wdram[et * 128:(et + 1) * 128, :])
                    nc.sync.dma_start(
                        WT[:, :, et * 128:(et + 1) * 128], strip[:], transpose=True
                    )

            def load_wo():
                for et8 in range(ND):
                    strip = stripp.tile([128, E], F16, tag="stripo", name="stripo")
                    nc.gpsimd.dma_start(strip[:], wo[et8 * 128:(et8 + 1) * 128, :])
                    nc.sync.dma_start(
                        WoT[:, :, et8 * 128:(et8 + 1) * 128], strip[:], transpose=True
                    )

            def load_x(lt):
                strip = stripp.tile([128, D], F16, tag="strip", name="strip")
                nc.gpsimd.dma_start(strip[:], xb[lt * 128:(lt + 1) * 128, :])
                nc.sync.dma_start(
                    xT[:, :, lt * 128:(lt + 1) * 128], strip[:], transpose=True
                )

            def k_proj_group(et, jc):
                """K^T chunk: stationary WTk tiles, moving xT."""
                def gen():
                    pp = psP.tile([128, 512], F32, tag="pp", name="pp")
                    for dt in range(ND):
                        yield lambda dt=dt, pp=pp: nc.tensor.matmul(
                            pp[:],
                            WTk[:, dt, et * 128:(et + 1) * 128],
                            xT[:, dt, jc * 512:(jc + 1) * 512],
                            start=(dt == 0),
                            stop=(dt == ND - 1),
                        )
                    yield lambda pp=pp: nc.vector.tensor_copy(
                        KT[:, et, jc * 512:(jc + 1) * 512], pp[:]
                    )
                return gen()

            def v_proj_group(lt):
                """V natural for one l(=k)-tile: stationary xT tiles, moving WTv.
                Output [k-tile 128, 512 = 8 heads x 64] lands in Vaug."""
                def gen():
                    pp = psP.tile([128, 512], F32, tag="pp", name="pp")
                    for dt in range(ND):
                        yield lambda dt=dt, pp=pp: nc.tensor.matmul(
                            pp[:],
                            xT[:, dt, lt * 128:(lt + 1) * 128],
                            WTv[:, dt, :],
                            start=(dt == 0),
                            stop=(dt == ND - 1),
                        )
                    yield lambda pp=pp: nc.vector.tensor_copy(
                        Vaug[:, lt, :, 0:64], pp[:]
                    )
                return gen()

            def q_proj_group(j, qtile, et):
                def gen():
                    pp = psP.tile([128, 512], F32, tag="pp", name="pp")
                    for dt in range(ND):
                        yield lambda dt=dt, pp=pp: nc.tensor.matmul(
                            pp[:],
                            WTq[:, dt, et * 128:(et + 1) * 128],
                            xT[:, dt, j * 512:(j + 1) * 512],
                            start=(dt == 0),
                            stop=(dt == ND - 1),
                        )
                    yield lambda pp=pp: nc.vector.tensor_copy(qtile[:, et, :], pp[:])
                return gen()

            def out_proj_group(lt, ec):
                def gen():
                    op = psP.tile([128, 512], F32, tag="pp", name="op")
                    for dt in range(NDO):
                        yield lambda dt=dt, op=op: nc.tensor.matmul(
                            op[:],
                            attT[:, dt, lt * 128:(lt + 1) * 128],
                            WoT[:, dt, ec * 512:(ec + 1) * 512],
                            start=(dt == 0),
                            stop=(dt == NDO - 1),
                        )
                    def tail(op=op):
                        ot = smallp.tile([128, 512], F32, tag="ot", name="ot")
                        nc.vector.tensor_copy(ot[:], op[:])
                        nc.sync.dma_start(
                            out[lt * 128:(lt + 1) * 128, ec * 512:(ec + 1) * 512],
                            ot[:],
                        )
                    yield tail
                return gen()

            def chain(gens):
                for g in gens:
                    yield from g

            def drain(it, n):
                k = 0
                for f in it:
                    f()
                    k += 1
                    if k >= n:
                        return

            # ---------- prologue: minimum to start chunk-0 compute ----------
            load_w(wk, WTk)
            load_w(wv, WTv)
            for lt in range(4):
                load_x(lt)
            load_w(wq, WTq)
            for et in range(NE):
                drain(k_proj_group(et, 0), 99)
            for lt in range(4):
                drain(v_proj_group(lt), 99)
            qtiles = {0: qtcp.tile([128, NE, 512], MM, tag="qt", name="qt0")}
            for et in range(NE):
                drain(q_proj_group(0, qtiles[0], et), 99)
            for lt in range(4, NL):
                load_x(lt)
            load_wo()

            # ---------- chunk loop with per-kt filler interleave ----------
            for j in range(NJ):
                gens = []
                if j + 1 < NJ:
                    qtiles[j + 1] = qtcp.tile(
                        [128, NE, 512], MM, tag="qt", name=f"qt{j+1}"
                    )
                    for et in range(NE):
                        gens.append(k_proj_group(et, j + 1))
                    for lt in range(4 * (j + 1), 4 * (j + 1) + 4):
                        gens.append(v_proj_group(lt))
                    for et in range(NE):
                        gens.append(q_proj_group(j + 1, qtiles[j + 1], et))
                if j >= 1:
                    for lt in range(4 * (j - 1), 4 * (j - 1) + 4):
                        for ec in range(2):
                            gens.append(out_proj_group(lt, ec))
                filler = chain(gens)
                # filler instruction count ~= (8+1)*12 + (4+1)*8 = 148 for middle
                # chunks; attention kt-steps this chunk = 8 heads * nkt
                nkt = 4 * (j + 1)
                steps = 8 * nkt
                n_fill_items = (9 * (NE + 4 + NE) if j + 1 < NJ else 0) + (
                    5 * 8 if j >= 1 else 0
                )
                QTc = qtiles[j]
                fill_acc = 0.0
                fill_done = 0
                step = 0
                for h in range(8):
                    hp = (h % 2) * 64
                    hb = h // 2
                    PT = ptp.tile([128, NK, 512], MM, tag="pt", name="pt")
                    for kt in range(nkt):
                        s_ps = psS.tile([128, 512], F32, tag="s", name="s_ps")
                        nc.tensor.matmul(
                            s_ps[:],
                            KT[hp:hp + 64, hb, kt * 128:(kt + 1) * 128],
                            QTc[hp:hp + 64, hb, :],
                        )
                        nc.scalar.activation(
                            PT[:, kt, :], s_ps[:], AF.Exp, scale=0.125
                        )
                        if kt >= nkt - 4:
                            nc.vector.tensor_mul(
                                PT[:, kt, :],
                                PT[:, kt, :],
                                masks_sb[:, kt - (nkt - 4), :],
                            )
                        att_ps = (
                            psAV.tile([65, 512], F32, tag="av", name="att_ps")
                            if kt == 0
                            else att_ps
                        )
                        nc.tensor.matmul(
                            att_ps[:],
                            Vaug[:, kt, h, 0:65],
                            PT[:, kt, :],
                            start=(kt == 0),
                            stop=(kt == nkt - 1),
                        )
                        step += 1
                        fill_acc += n_fill_items / steps
                        take = int(fill_acc) - fill_done
                        if take > 0:
                            drain(filler, take)
                            fill_done += take
                    recip = smallp.tile([1, 512], MM, tag="recip", name="recip")
                    with nc.allow_low_precision(reason="fp16 for PE broadcast"):
                        nc.vector.reciprocal(recip[:], att_ps[64:65, :])
                    bc_ps = psX.tile([64, 512], F32, tag="x", name="bc_ps")
                    nc.tensor.matmul(bc_ps[:], ones_r[:], recip[:])
                    bc_sb = smallp.tile([64, 512], F32, tag="bcsb", name="bc_sb")
                    nc.vector.tensor_copy(bc_sb[:], bc_ps[:])
                    nc.vector.tensor_mul(
                        attT[hp:hp + 64, hb, j * 512:(j + 1) * 512],
                        att_ps[0:64, :],
                        bc_sb[:],
                    )
                drain(filler, 10 ** 9)

            for lt in range(4 * (NJ - 1), 4 * (NJ - 1) + 4):
                for ec in range(2):
                    drain(out_proj_group(lt, ec), 99)

    nc.compile()
    return nc


def build_masks():
    kp = np.arange(128)[:, None]
    qf = np.arange(512)[None, :]
    return np.stack([(qf >= kp + 128 * m).astype(np.float16) for m in range(4)])


def _get_program():
    if "nc" not in _CACHE:
        _CACHE["nc"] = build_program()
    return _CACHE["nc"]


def make_in_maps(x, Wq, Wk, Wv, Wo):
    x = np.asarray(x, dtype=np.float32)
    Wq = np.asarray(Wq, dtype=np.float32)
    Wk = np.asarray(Wk, dtype=np.float32)
    Wv = np.asarray(Wv, dtype=np.float32)
    Wo = np.asarray(Wo, dtype=np.float32)
    masks = build_masks()
    in_maps = []
    for c in range(8):
        b, hg = c // 2, c % 2
        sl = slice(hg * E, (hg + 1) * E)
        in_maps.append(
            {
                "xb": np.ascontiguousarray(x[b]),
                "wq": np.ascontiguousarray(Wq[sl]),
                "wk": np.ascontiguousarray(Wk[sl]),
                "wv": np.ascontiguousarray(Wv[sl]),
                "wo": np.ascontiguousarray(Wo[:, sl]),
                "masks": masks,
            }
        )
    return in_maps


def kernel(x, Wq, Wk, Wv, Wo, **run_kwargs):
    from concourse import bass_utils

    nc = _get_program()
    in_maps = make_in_maps(x, Wq, Wk, Wv, Wo)
    res = bass_utils.run_bass_kernel_spmd(
        nc, in_maps, core_ids=list(range(8)), **run_kwargs
    )
    o = np.empty((B, L, D), np.float32)
    for b in range(B):
        o[b] = res.results[2 * b]["out"] + res.results[2 * b + 1]["out"]
    _CACHE["last_result"] = res
    return o


# revision 15
# speedup vs baseline: 1.1820x; 1.0648x over previous
"""Multi-head causal attention (B=4, L=2048, D=1024, H=16) on 8 TRN2 NeuronCores.

Sharding: core c handles batch b = c//2 and head-group hg = c%2 (8 heads, 512
dims). Each core computes Q/K/V projections for its heads, causal attention,
and a partial output projection (its 512 input dims of Wo). Host sums the two
partials per batch.

fp16 matmuls. The PE instruction stream is kept dense through the attention
phase by interleaving projection work (K/V/Q of the next q-chunk, output
projection of the previous q-chunk) between attention matmuls at fine grain —
otherwise the activity monitor drops the PE clock from 2.4 to 1.2 GHz.
"""
import sys

sys.path.insert(0, "/opt/trn_rl_repo")

import numpy as np

import concourse.bass as bass
import concourse.mybir as mybir
import concourse.tile as tile
from concourse import bacc
from concourse.masks import make_identity

F32 = mybir.dt.float32
F16 = mybir.dt.float16
MM = F16
AF = mybir.ActivationFunctionType

B, L, D, H = 4, 2048, 1024, 16
DK = 64
E = 512
NL = L // 128
ND = D // 128
NE = E // 128
NJ = L // 512
NK = L // 128
NDO = E // 128

PT_BUFS = 2

_CACHE = {}


def build_program():
    nc = bacc.Bacc("TRN2", target_bir_lowering=False, debug=False, num_devices=8)

    xb = nc.dram_tensor("xb", [L, D], F32, kind="ExternalInput")
    wq = nc.dram_tensor("wq", [E, D], F32, kind="ExternalInput")
    wk = nc.dram_tensor("wk", [E, D], F32, kind="ExternalInput")
    wv = nc.dram_tensor("wv", [E, D], F32, kind="ExternalInput")
    wo = nc.dram_tensor("wo", [D, E], F32, kind="ExternalInput")
    masks = nc.dram_tensor("masks", [4, 128, 512], F16, kind="ExternalInput")
    out = nc.dram_tensor("out", [L, D], F32, kind="ExternalOutput")

    with tile.TileContext(nc) as tc:
        with (
            tc.tile_pool(name="const", bufs=1) as constp,
            tc.tile_pool(name="big", bufs=1) as bigp,
            tc.tile_pool(name="stripp", bufs=3) as stripp,
            tc.tile_pool(name="qtc", bufs=2) as qtcp,
            tc.tile_pool(name="ptp", bufs=PT_BUFS) as ptp,
            tc.tile_pool(name="smallp", bufs=2) as smallp,
            tc.tile_pool(name="psX", bufs=1, space="PSUM") as psX,
            tc.tile_pool(name="psP", bufs=2, space="PSUM") as psP,
            tc.tile_pool(name="psS", bufs=3, space="PSUM") as psS,
            tc.tile_pool(name="psAV", bufs=2, space="PSUM") as psAV,
        ):
            ident = constp.tile([128, 128], F32)
            make_identity(nc, ident[:])
            ones_f = constp.tile([128, 16], F32)
            nc.vector.memset(ones_f[:], 1.0)
            ones_r = constp.tile([1, 64], MM)
            nc.vector.tensor_copy(ones_r[:], ones_f[0:1, 0:1].to_broadcast((1, 64)))
            masks_sb = constp.tile([128, 4, 512], F16)
            for m in range(4):
                nc.sync.dma_start(masks_sb[:, m, :], masks[m])

            xT = bigp.tile([128, ND, L], MM)       # x^T  [d-in-tile, d-tile, l]
            WTq = bigp.tile([128, ND, E], MM)      # Wq^T [d-in-tile, d-tile, e]
            WTk = bigp.tile([128, ND, E], MM)
            WTv = bigp.tile([128, ND, E], MM)
            KT = bigp.tile([128, NE, L], MM)       # K^T  [dk (2 heads), e-tile, k]
            attT = bigp.tile([128, NDO, L], MM)
            WoT = bigp.tile([128, NDO, D], MM)
            Vaug = bigp.tile([128, NK, 8, 65], MM)  # V natural per (k-tile, head) + ones

            nc.vector.tensor_copy(
                Vaug[:, :, :, 64:65],
                ones_f[:, 0:1].to_broadcast((128, NK, 8, 1)),
            )

            # ---------- emission helpers ----------
            def transpose_strip(dst, strip, ncols):
                for dt in range(ncols):
                    tp = psX.tile([128, 128], F32, tag="x", name="tp")
                    nc.tensor.transpose(
                        tp[:], strip[:, dt * 128:(dt + 1) * 128], ident[:]
                    )
                    nc.vector.tensor_copy(dst[:, dt, :], tp[:])

            def load_w(wdram, WT):
                for et in range(WT.shape[2] // 128):
                    strip = stripp.tile([128, D], F32, tag="strip", name="strip")
                    nc.sync.dma_start(strip[:], wdram[et * 128:(et + 1) * 128, :])
                    transpose_strip(WT[:, :, et * 128:(et + 1) * 128], strip, ND)

            def load_wo():
                for et8 in range(ND):
                    strip = stripp.tile([128, E], F32, tag="stripo", name="stripo")
                    nc.sync.dma_start(strip[:], wo[et8 * 128:(et8 + 1) * 128, :])
                    transpose_strip(WoT[:, :, et8 * 128:(et8 + 1) * 128], strip, NDO)

            def load_x(lt):
                strip = stripp.tile([128, D], F32, tag="strip", name="strip")
                nc.sync.dma_start(strip[:], xb[lt * 128:(lt + 1) * 128, :])
                transpose_strip(xT[:, :, lt * 128:(lt + 1) * 128], strip, ND)

            def k_proj_group(et, jc):
                """K^T chunk: stationary WTk tiles, moving xT."""
                def gen():
                    pp = psP.tile([128, 512], F32, tag="pp", name="pp")
                    for dt in range(ND):
                        yield lambda dt=dt, pp=pp: nc.tensor.matmul(
                            pp[:],
                            WTk[:, dt, et * 128:(et + 1) * 128],
                            xT[:, dt, jc * 512:(jc + 1) * 512],
                            start=(dt == 0),
                            stop=(dt == ND - 1),
                        )
                    yield lambda pp=pp: nc.vector.tensor_copy(
                        KT[:, et, jc * 512:(jc + 1) * 512], pp[:]
                    )
                return gen()

            def v_proj_group(lt):
                """V natural for one l(=k)-tile: stationary xT tiles, moving WTv.
                Output [k-tile 128, 512 = 8 heads x 64] lands in Vaug."""
                def gen():
                    pp = psP.tile([128, 512], F32, tag="pp", name="pp")
                    for dt in range(ND):
                        yield lambda dt=dt, pp=pp: nc.tensor.matmul(
                            pp[:],
                            xT[:, dt, lt * 128:(lt + 1) * 128],
                            WTv[:, dt, :],
                            start=(dt == 0),
                            stop=(dt == ND - 1),
                        )
                    yield lambda pp=pp: nc.vector.tensor_copy(
                        Vaug[:, lt, :, 0:64], pp[:]
                    )
                return gen()

            def q_proj_group(j, qtile, et):
                def gen():
                    pp = psP.tile([128, 512], F32, tag="pp", name="pp")
                    for dt in range(ND):
                        yield lambda dt=dt, pp=pp: nc.tensor.matmul(
                            pp[:],
                            WTq[:, dt, et * 128:(et + 1) * 128],
                            xT[:, dt, j * 512:(j + 1) * 512],
                            start=(dt == 0),
                            stop=(dt == ND - 1),
                        )
                    yield lambda pp=pp: nc.vector.tensor_copy(qtile[:, et, :], pp[:])
                return gen()

            def out_proj_group(lt, ec):
                def gen():
                    op = psP.tile([128, 512], F32, tag="pp", name="op")
                    for dt in range(NDO):
                        yield lambda dt=dt, op=op: nc.tensor.matmul(
                            op[:],
                            attT[:, dt, lt * 128:(lt + 1) * 128],
                            WoT[:, dt, ec * 512:(ec + 1) * 512],
                            start=(dt == 0),
                            stop=(dt == NDO - 1),
                        )
                    def tail(op=op):
                        ot = smallp.tile([128, 512], F32, tag="ot", name="ot")
                        nc.vector.tensor_copy(ot[:], op[:])
                        nc.sync.dma_start(
                            out[lt * 128:(lt + 1) * 128, ec * 512:(ec + 1) * 512],
                            ot[:],
                        )
                    yield tail
                return gen()

            def chain(gens):
                for g in gens:
                    yield from g

            def drain(it, n):
                k = 0
                for f in it:
                    f()
                    k += 1
                    if k >= n:
                        return

            # ---------- prologue: minimum to start chunk-0 compute ----------
            load_w(wk, WTk)
            load_w(wv, WTv)
            for lt in range(4):
                load_x(lt)
            load_w(wq, WTq)
            for et in range(NE):
                drain(k_proj_group(et, 0), 99)
            for lt in range(4):
                drain(v_proj_group(lt), 99)
            qtiles = {0: qtcp.tile([128, NE, 512], MM, tag="qt", name="qt0")}
            for et in range(NE):
                drain(q_proj_group(0, qtiles[0], et), 99)
            for lt in range(4, NL):
                load_x(lt)
            load_wo()

            # ---------- chunk loop with per-kt filler interleave ----------
            for j in range(NJ):
                gens = []
                if j + 1 < NJ:
                    qtiles[j + 1] = qtcp.tile(
                        [128, NE, 512], MM, tag="qt", name=f"qt{j+1}"
                    )
                    for et in range(NE):
                        gens.append(k_proj_group(et, j + 1))
                    for lt in range(4 * (j + 1), 4 * (j + 1) + 4):
                        gens.append(v_proj_group(lt))
                    for et in range(NE):
                        gens.append(q_proj_group(j + 1, qtiles[j + 1], et))
                if j >= 1:
                    for lt in range(4 * (j - 1), 4 * (j - 1) + 4):
                        for ec in range(2):
                            gens.append(out_proj_group(lt, ec))
                filler = chain(gens)
                # filler instruction count ~= (8+1)*12 + (4+1)*8 = 148 for middle
                # chunks; attention kt-steps this chunk = 8 heads * nkt
                nkt = 4 * (j + 1)
                steps = 8 * nkt
                n_fill_items = (9 * (NE + 4 + NE) if j + 1 < NJ else 0) + (
                    5 * 8 if j >= 1 else 0
                )
                QTc = qtiles[j]
                fill_acc = 0.0
                fill_done = 0
                step = 0
                for h in range(8):
                    hp = (h % 2) * 64
                    hb = h // 2
                    PT = ptp.tile([128, NK, 512], MM, tag="pt", name="pt")
                    for kt in range(nkt):
                        s_ps = psS.tile([128, 512], F32, tag="s", name="s_ps")
                        nc.tensor.matmul(
                            s_ps[:],
                            KT[hp:hp + 64, hb, kt * 128:(kt + 1) * 128],
                            QTc[hp:hp + 64, hb, :],
                        )
                        nc.scalar.activation(
                            PT[:, kt, :], s_ps[:], AF.Exp, scale=0.125
                        )
                        if kt >= nkt - 4:
                            nc.vector.tensor_mul(
                                PT[:, kt, :],
                                PT[:, kt, :],
                                masks_sb[:, kt - (nkt - 4), :],
                            )
                        att_ps = (
                            psAV.tile([65, 512], F32, tag="av", name="att_ps")
                            if kt == 0
                            else att_ps
                        )
                        nc.tensor.matmul(
                            att_ps[:],
                            Vaug[:, kt, h, 0:65],
                            PT[:, kt, :],
                            start=(kt == 0),
                            stop=(kt == nkt - 1),
                        )
                        step += 1
                        fill_acc += n_fill_items / steps
                        take = int(fill_acc) - fill_done
                        if take > 0:
                            drain(filler, take)
                            fill_done += take
                    denom = smallp.tile([1, 512], MM, tag="recip", name="denom")
                    with nc.allow_low_precision(reason="fp16 for PE broadcast"):
                        nc.vector.tensor_copy(denom[:], att_ps[64:65, :])
                    bc_ps = psX.tile([64, 512], F32, tag="x", name="bc_ps")
                    nc.tensor.matmul(bc_ps[:], ones_r[:], denom[:])
                    bc_sb = smallp.tile([64, 512], F32, tag="bcsb", name="bc_sb")
                    nc.vector.reciprocal(bc_sb[:], bc_ps[:])
                    nc.vector.tensor_mul(
                        attT[hp:hp + 64, hb, j * 512:(j + 1) * 512],
                        att_ps[0:64, :],
                        bc_sb[:],
                    )
                drain(filler, 10 ** 9)

            for lt in range(4 * (NJ - 1), 4 * (NJ - 1) + 4):
                for ec in range(2):
                    drain(out_proj_group(lt, ec), 99)

    nc.compile()
    return nc


def build_masks():
    kp = np.arange(128)[:, None]
    qf = np.arange(512)[None, :]
    return np.stack([(qf >= kp + 128 * m).astype(np.float16) for m in range(4)])


def _get_program():
    if "nc" not in _CACHE:
        _CACHE["nc"] = build_program()
    return _CACHE["nc"]


def make_in_maps(x, Wq, Wk, Wv, Wo):
    x = np.asarray(x, dtype=np.float32)
    Wq = np.asarray(Wq, dtype=np.float32)
    Wk = np.asarray(Wk, dtype=np.float32)
    Wv = np.asarray(Wv, dtype=np.float32)
    Wo = np.asarray(Wo, dtype=np.float32)
    masks = build_masks()
    in_maps = []
    for c in range(8):
        b, hg = c // 2, c % 2
        sl = slice(hg * E, (hg + 1) * E)
        in_maps.append(
            {
                "xb": np.ascontiguousarray(x[b]),
                "wq": np.ascontiguousarray(Wq[sl]),
                "wk": np.ascontiguousarray(Wk[sl]),
                "wv": np.ascontiguousarray(Wv[sl]),
                "wo": np.ascontiguousarray(Wo[:, sl]),
                "masks": masks,
            }
        )
    return in_maps


def kernel(x, Wq, Wk, Wv, Wo, **run_kwargs):
    from concourse import bass_utils

    nc = _get_program()
    in_maps = make_in_maps(x, Wq, Wk, Wv, Wo)
    res = bass_utils.run_bass_kernel_spmd(
        nc, in_maps, core_ids=list(range(8)), **run_kwargs
    )
    o = np.empty((B, L, D), np.float32)
    for b in range(B):
        o[b] = res.results[2 * b]["out"] + res.results[2 * b + 1]["out"]
    _CACHE["last_result"] = res
    return o


# revision 16
# speedup vs baseline: 1.7945x; 1.5182x over previous
"""Multi-head causal attention (B=4, L=2048, D=1024, H=16) on 8 TRN2 NeuronCores.

Sharding: core c handles batch b = c//2 and head-group hg = c%2 (8 heads, 512
dims). Each core computes Q/K/V projections for its heads, causal attention,
and a partial output projection (its 512 input dims of Wo). Host sums the two
partials per batch.

fp16 matmuls. The PE instruction stream is kept dense through the attention
phase by smearing projection work (K/V/Q of the next q-chunk; all output
projection deferred into the last, filler-starved chunk) between attention
matmuls — otherwise the activity monitor drops the PE clock to 1.2 GHz.
AV runs in the att-natural orientation (P^T stationary, V+ones moving) so the
softmax denominator lands per-partition: reciprocal is a [128,1] op and the
division a native tensor_scalar multiply.
"""
import sys

sys.path.insert(0, "/opt/trn_rl_repo")

import numpy as np

import concourse.bass as bass
import concourse.mybir as mybir
import concourse.tile as tile
from concourse import bacc
from concourse.masks import make_identity

F32 = mybir.dt.float32
F16 = mybir.dt.float16
MM = F16
AF = mybir.ActivationFunctionType

B, L, D, H = 4, 2048, 1024, 16
DK = 64
E = 512
NL = L // 128
ND = D // 128
NE = E // 128
NJ = L // 512
NK = L // 128
NDO = E // 128

PT_BUFS = 2

_CACHE = {}


def build_program():
    nc = bacc.Bacc("TRN2", target_bir_lowering=False, debug=False, num_devices=8)

    xb = nc.dram_tensor("xb", [L, D], F32, kind="ExternalInput")
    wq = nc.dram_tensor("wq", [E, D], F32, kind="ExternalInput")
    wk = nc.dram_tensor("wk", [E, D], F32, kind="ExternalInput")
    wv = nc.dram_tensor("wv", [E, D], F32, kind="ExternalInput")
    wo = nc.dram_tensor("wo", [D, E], F32, kind="ExternalInput")
    masks = nc.dram_tensor("masks", [4, 128, 512], F16, kind="ExternalInput")
    out = nc.dram_tensor("out", [L, D], F32, kind="ExternalOutput")

    with tile.TileContext(nc) as tc:
        with (
            tc.tile_pool(name="const", bufs=1) as constp,
            tc.tile_pool(name="big", bufs=1) as bigp,
            tc.tile_pool(name="stripp", bufs=3) as stripp,
            tc.tile_pool(name="qtc", bufs=2) as qtcp,
            tc.tile_pool(name="ptp", bufs=PT_BUFS) as ptp,
            tc.tile_pool(name="smallp", bufs=2) as smallp,
            tc.tile_pool(name="attsbp", bufs=2) as attsbp,
            tc.tile_pool(name="psX", bufs=2, space="PSUM") as psX,
            tc.tile_pool(name="psP", bufs=2, space="PSUM") as psP,
            tc.tile_pool(name="psS", bufs=2, space="PSUM") as psS,
            tc.tile_pool(name="psAV", bufs=2, space="PSUM") as psAV,
        ):
            ident = constp.tile([128, 128], F32)
            make_identity(nc, ident[:])
            ident_h = constp.tile([128, 128], F16)
            make_identity(nc, ident_h[:])
            ones_f = constp.tile([128, 16], F32)
            nc.vector.memset(ones_f[:], 1.0)
            masks_sb = constp.tile([128, 4, 512], F16)
            for m in range(4):
                nc.sync.dma_start(masks_sb[:, m, :], masks[m])

            xT = bigp.tile([128, ND, L], MM)       # x^T  [d-in-tile, d-tile, l]
            WTq = bigp.tile([128, ND, E], MM)      # Wq^T [d-in-tile, d-tile, e]
            WTk = bigp.tile([128, ND, E], MM)
            WTv = bigp.tile([128, ND, E], MM)
            KT = bigp.tile([128, NE, L], MM)       # K^T  [dk (2 heads), e-tile, k]
            attT = bigp.tile([128, NDO, L], MM)
            WoT = bigp.tile([128, NDO, D], MM)
            Vaug = bigp.tile([128, NK, 8, 65], MM)  # V natural per (k-tile, head) + ones

            nc.vector.tensor_copy(
                Vaug[:, :, :, 64:65],
                ones_f[:, 0:1].to_broadcast((128, NK, 8, 1)),
            )

            # ---------- emission helpers ----------
            def transpose_strip(dst, strip, ncols):
                for dt in range(ncols):
                    tp = psX.tile([128, 128], F32, tag="x", name="tp")
                    nc.tensor.transpose(
                        tp[:], strip[:, dt * 128:(dt + 1) * 128], ident[:]
                    )
                    nc.vector.tensor_copy(dst[:, dt, :], tp[:])

            def load_w(wdram, WT):
                for et in range(WT.shape[2] // 128):
                    strip = stripp.tile([128, D], F32, tag="strip", name="strip")
                    nc.sync.dma_start(strip[:], wdram[et * 128:(et + 1) * 128, :])
                    transpose_strip(WT[:, :, et * 128:(et + 1) * 128], strip, ND)

            def load_wo():
                for et8 in range(ND):
                    strip = stripp.tile([128, E], F32, tag="stripo", name="stripo")
                    nc.sync.dma_start(strip[:], wo[et8 * 128:(et8 + 1) * 128, :])
                    transpose_strip(WoT[:, :, et8 * 128:(et8 + 1) * 128], strip, NDO)

            def load_x(lt):
                strip = stripp.tile([128, D], F32, tag="strip", name="strip")
                nc.sync.dma_start(strip[:], xb[lt * 128:(lt + 1) * 128, :])
                transpose_strip(xT[:, :, lt * 128:(lt + 1) * 128], strip, ND)

            def k_proj_group(et, jc):
                def gen():
                    pp = psP.tile([128, 512], F32, tag="pp", name="pp")
                    for dt in range(ND):
                        yield lambda dt=dt, pp=pp: nc.tensor.matmul(
                            pp[:],
                            WTk[:, dt, et * 128:(et + 1) * 128],
                            xT[:, dt, jc * 512:(jc + 1) * 512],
                            start=(dt == 0),
                            stop=(dt == ND - 1),
                        )
                    yield lambda pp=pp: nc.vector.tensor_copy(
                        KT[:, et, jc * 512:(jc + 1) * 512], pp[:]
                    )
                return gen()

            def v_proj_group(lt):
                def gen():
                    pp = psP.tile([128, 512], F32, tag="pp", name="pp")
                    for dt in range(ND):
                        yield lambda dt=dt, pp=pp: nc.tensor.matmul(
                            pp[:],
                            xT[:, dt, lt * 128:(lt + 1) * 128],
                            WTv[:, dt, :],
                            start=(dt == 0),
                            stop=(dt == ND - 1),
                        )
                    yield lambda pp=pp: nc.vector.tensor_copy(
                        Vaug[:, lt, :, 0:64], pp[:]
                    )
                return gen()

            def q_proj_group(j, qtile, et):
                def gen():
                    pp = psP.tile([128, 512], F32, tag="pp", name="pp")
                    for dt in range(ND):
                        yield lambda dt=dt, pp=pp: nc.tensor.matmul(
                            pp[:],
                            WTq[:, dt, et * 128:(et + 1) * 128],
                            xT[:, dt, j * 512:(j + 1) * 512],
                            start=(dt == 0),
                            stop=(dt == ND - 1),
                        )
                    yield lambda pp=pp: nc.vector.tensor_copy(qtile[:, et, :], pp[:])
                return gen()

            def out_proj_group(lt, ec):
                def gen():
                    op = psP.tile([128, 512], F32, tag="pp", name="op")
                    for dt in range(NDO):
                        yield lambda dt=dt, op=op: nc.tensor.matmul(
                            op[:],
                            attT[:, dt, lt * 128:(lt + 1) * 128],
                            WoT[:, dt, ec * 512:(ec + 1) * 512],
                            start=(dt == 0),
                            stop=(dt == NDO - 1),
                        )
                    def tail(op=op):
                        ot = smallp.tile([128, 512], F32, tag="ot", name="ot")
                        nc.vector.tensor_copy(ot[:], op[:])
                        nc.sync.dma_start(
                            out[lt * 128:(lt + 1) * 128, ec * 512:(ec + 1) * 512],
                            ot[:],
                        )
                    yield tail
                return gen()

            def chain(gens):
                for g in gens:
                    yield from g

            def drain(it, n):
                k = 0
                for f in it:
                    f()
                    k += 1
                    if k >= n:
                        return

            # ---------- prologue ----------
            load_w(wk, WTk)
            load_w(wv, WTv)
            for lt in range(4):
                load_x(lt)
            load_w(wq, WTq)
            for et in range(NE):
                drain(k_proj_group(et, 0), 99)
            for lt in range(4):
                drain(v_proj_group(lt), 99)
            qtiles = {0: qtcp.tile([128, NE, 512], MM, tag="qt", name="qt0")}
            for et in range(NE):
                drain(q_proj_group(0, qtiles[0], et), 99)
            for lt in range(4, NL):
                load_x(lt)
            load_wo()

            # ---------- chunk loop ----------
            for j in range(NJ):
                gens = []
                n_fill_items = 0
                if j + 1 < NJ:
                    qtiles[j + 1] = qtcp.tile(
                        [128, NE, 512], MM, tag="qt", name=f"qt{j+1}"
                    )
                    for et in range(NE):
                        gens.append(k_proj_group(et, j + 1))
                    for lt in range(4 * (j + 1), 4 * (j + 1) + 4):
                        gens.append(v_proj_group(lt))
                    for et in range(NE):
                        gens.append(q_proj_group(j + 1, qtiles[j + 1], et))
                    n_fill_items += 9 * 12
                else:
                    # chunk 3 has no next-chunk projections: feed it ALL the
                    # output projection of chunks 0-2
                    for lt in range(0, 12):
                        for ec in range(2):
                            gens.append(out_proj_group(lt, ec))
                    n_fill_items += 5 * 24
                filler = chain(gens)
                nkt = 4 * (j + 1)
                steps = 8 * nkt
                QTc = qtiles[j]
                fill_acc = 0.0
                fill_done = 0
                for h in range(8):
                    hp = (h % 2) * 64
                    hb = h // 2
                    if h % 2 == 0:
                        att_sbs = [
                            attsbp.tile([128, 128], MM, tag=f"asb{qt}", name=f"asb{qt}")
                            for qt in range(4)
                        ]
                    PT = ptp.tile([128, NK, 512], MM, tag="pt", name="pt")
                    for kt in range(nkt):
                        s_ps = psS.tile([128, 512], F32, tag="s", name="s_ps")
                        nc.tensor.matmul(
                            s_ps[:],
                            KT[hp:hp + 64, hb, kt * 128:(kt + 1) * 128],
                            QTc[hp:hp + 64, hb, :],
                        )
                        nc.scalar.activation(
                            PT[:, kt, :], s_ps[:], AF.Exp, scale=0.125
                        )
                        if kt >= nkt - 4:
                            nc.vector.tensor_mul(
                                PT[:, kt, :],
                                PT[:, kt, :],
                                masks_sb[:, kt - (nkt - 4), :],
                            )
                        fill_acc += n_fill_items / (steps * 1.2)
                        take = int(fill_acc) - fill_done
                        if take > 0:
                            drain(filler, take)
                            fill_done += take
                    for qt in range(4):
                        att_ps = psAV.tile([128, 65], F32, tag="av", name="att_ps")
                        nq = 4 * j + qt + 1
                        for kt in range(nq):
                            nc.tensor.matmul(
                                att_ps[:],
                                PT[:, kt, qt * 128:(qt + 1) * 128],
                                Vaug[:, kt, h, 0:65],
                                start=(kt == 0),
                                stop=(kt == nq - 1),
                            )
                        rc = smallp.tile([128, 1], F32, tag="rc", name="rc")
                        nc.vector.reciprocal(rc[:], att_ps[:, 64:65])
                        nc.vector.tensor_scalar_mul(
                            att_sbs[qt][:, hp:hp + 64], att_ps[:, 0:64], rc[:]
                        )
                        fill_acc += n_fill_items / (steps * 3.0)
                        take = int(fill_acc) - fill_done
                        if take > 0:
                            drain(filler, take)
                            fill_done += take
                    if h % 2 == 1:
                        for qt in range(4):
                            tpa = psX.tile([128, 128], F16, tag="x", name="tpa")
                            nc.tensor.transpose(tpa[:], att_sbs[qt][:], ident_h[:])
                            nc.vector.tensor_copy(
                                attT[:, hb, j * 512 + qt * 128:j * 512 + (qt + 1) * 128],
                                tpa[:],
                            )
                drain(filler, 10 ** 9)

            for lt in range(4 * (NJ - 1), 4 * (NJ - 1) + 4):
                for ec in range(2):
                    drain(out_proj_group(lt, ec), 99)

    nc.compile()
    return nc


def build_masks():
    kp = np.arange(128)[:, None]
    qf = np.arange(512)[None, :]
    return np.stack([(qf >= kp + 128 * m).astype(np.float16) for m in range(4)])


def _get_program():
    if "nc" not in _CACHE:
        _CACHE["nc"] = build_program()
    return _CACHE["nc"]


def make_in_maps(x, Wq, Wk, Wv, Wo):
    x = np.asarray(x, dtype=np.float32)
    Wq = np.asarray(Wq, dtype=np.float32)
    Wk = np.asarray(Wk, dtype=np.float32)
    Wv = np.asarray(Wv, dtype=np.float32)
    Wo = np.asarray(Wo, dtype=np.float32)
    masks = build_masks()
    in_maps = []
    for c in range(8):
        b, hg = c // 2, c % 2
        sl = slice(hg * E, (hg + 1) * E)
        in_maps.append(
            {
                "xb": np.ascontiguousarray(x[b]),
                "wq": np.ascontiguousarray(Wq[sl]),
                "wk": np.ascontiguousarray(Wk[sl]),
                "wv": np.ascontiguousarray(Wv[sl]),
                "wo": np.ascontiguousarray(Wo[:, sl]),
                "masks": masks,
            }
        )
    return in_maps


def kernel(x, Wq, Wk, Wv, Wo, **run_kwargs):
    from concourse import bass_utils

    nc = _get_program()
    in_maps = make_in_maps(x, Wq, Wk, Wv, Wo)
    res = bass_utils.run_bass_kernel_spmd(
        nc, in_maps, core_ids=list(range(8)), **run_kwargs
    )
    o = np.empty((B, L, D), np.float32)
    for b in range(B):
        o[b] = res.results[2 * b]["out"] + res.results[2 * b + 1]["out"]
    _CACHE["last_result"] = res
    return o


# revision 17
# speedup vs baseline: 1.8226x; 1.0156x over previous
"""Multi-head causal attention (B=4, L=2048, D=1024, H=16) on 8 TRN2 NeuronCores.

Sharding: core c handles batch b = c//2 and head-group hg = c%2 (8 heads, 512
dims). Each core computes Q/K/V projections for its heads, causal attention,
and a partial output projection (its 512 input dims of Wo). Host sums the two
partials per batch.

fp16 matmuls. The PE instruction stream is kept dense through the attention
phase by smearing projection work (K/V/Q of the next q-chunk; all output
projection deferred into the last, filler-starved chunk) between attention
matmuls — otherwise the activity monitor drops the PE clock to 1.2 GHz.
AV runs in the att-natural orientation (P^T stationary, V+ones moving) so the
softmax denominator lands per-partition: reciprocal is a [128,1] op and the
division a native tensor_scalar multiply.
"""
import sys

sys.path.insert(0, "/opt/trn_rl_repo")

import numpy as np

import concourse.bass as bass
import concourse.mybir as mybir
import concourse.tile as tile
from concourse import bacc
from concourse.masks import make_identity

F32 = mybir.dt.float32
F16 = mybir.dt.float16
MM = F16
AF = mybir.ActivationFunctionType

B, L, D, H = 4, 2048, 1024, 16
DK = 64
E = 512
NL = L // 128
ND = D // 128
NE = E // 128
NJ = L // 512
NK = L // 128
NDO = E // 128

PT_BUFS = 2

_CACHE = {}


def build_program():
    nc = bacc.Bacc("TRN2", target_bir_lowering=False, debug=False, num_devices=8)

    xb = nc.dram_tensor("xb", [L, D], F32, kind="ExternalInput")
    wq = nc.dram_tensor("wq", [E, D], F32, kind="ExternalInput")
    wk = nc.dram_tensor("wk", [E, D], F32, kind="ExternalInput")
    wv = nc.dram_tensor("wv", [E, D], F32, kind="ExternalInput")
    wo = nc.dram_tensor("wo", [D, E], F32, kind="ExternalInput")
    masks = nc.dram_tensor("masks", [4, 128, 512], F16, kind="ExternalInput")
    out = nc.dram_tensor("out", [L, D], F32, kind="ExternalOutput")

    with tile.TileContext(nc) as tc:
        with (
            tc.tile_pool(name="const", bufs=1) as constp,
            tc.tile_pool(name="big", bufs=1) as bigp,
            tc.tile_pool(name="stripp", bufs=3) as stripp,
            tc.tile_pool(name="qtc", bufs=2) as qtcp,
            tc.tile_pool(name="ptp", bufs=PT_BUFS) as ptp,
            tc.tile_pool(name="smallp", bufs=2) as smallp,
            tc.tile_pool(name="attsbp", bufs=2) as attsbp,
            tc.tile_pool(name="psX", bufs=2, space="PSUM") as psX,
            tc.tile_pool(name="psP", bufs=2, space="PSUM") as psP,
            tc.tile_pool(name="psS", bufs=2, space="PSUM") as psS,
            tc.tile_pool(name="psAV", bufs=2, space="PSUM") as psAV,
        ):
            ident = constp.tile([128, 128], F32)
            make_identity(nc, ident[:])
            ident_h = constp.tile([128, 128], F16)
            make_identity(nc, ident_h[:])
            ones_f = constp.tile([128, 16], F32)
            nc.vector.memset(ones_f[:], 1.0)
            masks_sb = constp.tile([128, 4, 512], F16)
            for m in range(4):
                nc.sync.dma_start(masks_sb[:, m, :], masks[m])

            xT = bigp.tile([128, ND, L], MM)       # x^T  [d-in-tile, d-tile, l]
            WTq = bigp.tile([128, ND, E], MM)      # Wq^T [d-in-tile, d-tile, e]
            WTk = bigp.tile([128, ND, E], MM)
            WTv = bigp.tile([128, ND, E], MM)
            KT = bigp.tile([128, NE, L], MM)       # K^T  [dk (2 heads), e-tile, k]
            attT = bigp.tile([128, NDO, L], MM)
            WoT = bigp.tile([128, NDO, D], MM)
            Vaug = bigp.tile([128, NK, 8, 65], MM)  # V natural per (k-tile, head) + ones

            nc.vector.tensor_copy(
                Vaug[:, :, :, 64:65],
                ones_f[:, 0:1].to_broadcast((128, NK, 8, 1)),
            )

            # ---------- emission helpers ----------
            def transpose_strip(dst, strip, ncols):
                for dt in range(ncols):
                    tp = psX.tile([128, 128], F32, tag="x", name="tp")
                    nc.tensor.transpose(
                        tp[:], strip[:, dt * 128:(dt + 1) * 128], ident[:]
                    )
                    nc.vector.tensor_copy(dst[:, dt, :], tp[:])

            def load_w(wdram, WT):
                for et in range(WT.shape[2] // 128):
                    strip = stripp.tile([128, D], F32, tag="strip", name="strip")
                    nc.sync.dma_start(strip[:], wdram[et * 128:(et + 1) * 128, :])
                    transpose_strip(WT[:, :, et * 128:(et + 1) * 128], strip, ND)

            def load_wo():
                for et8 in range(ND):
                    strip = stripp.tile([128, E], F32, tag="stripo", name="stripo")
                    nc.sync.dma_start(strip[:], wo[et8 * 128:(et8 + 1) * 128, :])
                    transpose_strip(WoT[:, :, et8 * 128:(et8 + 1) * 128], strip, NDO)

            def load_x(lt):
                strip = stripp.tile([128, D], F32, tag="strip", name="strip")
                nc.sync.dma_start(strip[:], xb[lt * 128:(lt + 1) * 128, :])
                transpose_strip(xT[:, :, lt * 128:(lt + 1) * 128], strip, ND)

            def k_proj_group(et, jc):
                def gen():
                    pp = psP.tile([128, 512], F32, tag="pp", name="pp")
                    for dt in range(ND):
                        yield lambda dt=dt, pp=pp: nc.tensor.matmul(
                            pp[:],
                            WTk[:, dt, et * 128:(et + 1) * 128],
                            xT[:, dt, jc * 512:(jc + 1) * 512],
                            start=(dt == 0),
                            stop=(dt == ND - 1),
                        )
                    yield lambda pp=pp: nc.vector.tensor_scalar_mul(
                        KT[:, et, jc * 512:(jc + 1) * 512], pp[:], 0.125
                    )
                return gen()

            def v_proj_group(lt):
                def gen():
                    pp = psP.tile([128, 512], F32, tag="pp", name="pp")
                    for dt in range(ND):
                        yield lambda dt=dt, pp=pp: nc.tensor.matmul(
                            pp[:],
                            xT[:, dt, lt * 128:(lt + 1) * 128],
                            WTv[:, dt, :],
                            start=(dt == 0),
                            stop=(dt == ND - 1),
                        )
                    yield lambda pp=pp: nc.vector.tensor_copy(
                        Vaug[:, lt, :, 0:64], pp[:]
                    )
                return gen()

            def q_proj_group(j, qtile, et):
                def gen():
                    pp = psP.tile([128, 512], F32, tag="pp", name="pp")
                    for dt in range(ND):
                        yield lambda dt=dt, pp=pp: nc.tensor.matmul(
                            pp[:],
                            WTq[:, dt, et * 128:(et + 1) * 128],
                            xT[:, dt, j * 512:(j + 1) * 512],
                            start=(dt == 0),
                            stop=(dt == ND - 1),
                        )
                    yield lambda pp=pp: nc.vector.tensor_copy(qtile[:, et, :], pp[:])
                return gen()

            def out_proj_group(lt, ec):
                def gen():
                    op = psP.tile([128, 512], F32, tag="pp", name="op")
                    for dt in range(NDO):
                        yield lambda dt=dt, op=op: nc.tensor.matmul(
                            op[:],
                            attT[:, dt, lt * 128:(lt + 1) * 128],
                            WoT[:, dt, ec * 512:(ec + 1) * 512],
                            start=(dt == 0),
                            stop=(dt == NDO - 1),
                        )
                    def tail(op=op):
                        ot = smallp.tile([128, 512], F32, tag="ot", name="ot")
                        nc.vector.tensor_copy(ot[:], op[:])
                        nc.sync.dma_start(
                            out[lt * 128:(lt + 1) * 128, ec * 512:(ec + 1) * 512],
                            ot[:],
                        )
                    yield tail
                return gen()

            def chain(gens):
                for g in gens:
                    yield from g

            def drain(it, n):
                k = 0
                for f in it:
                    f()
                    k += 1
                    if k >= n:
                        return

            # ---------- prologue ----------
            load_w(wk, WTk)
            load_w(wv, WTv)
            for lt in range(4):
                load_x(lt)
            load_w(wq, WTq)
            for et in range(NE):
                drain(k_proj_group(et, 0), 99)
            for lt in range(4):
                drain(v_proj_group(lt), 99)
            qtiles = {0: qtcp.tile([128, NE, 512], MM, tag="qt", name="qt0")}
            for et in range(NE):
                drain(q_proj_group(0, qtiles[0], et), 99)
            for lt in range(4, NL):
                load_x(lt)
            load_wo()

            # ---------- chunk loop ----------
            for j in range(NJ):
                gens = []
                n_fill_items = 0
                if j + 1 < NJ:
                    qtiles[j + 1] = qtcp.tile(
                        [128, NE, 512], MM, tag="qt", name=f"qt{j+1}"
                    )
                    for et in range(NE):
                        gens.append(k_proj_group(et, j + 1))
                    for lt in range(4 * (j + 1), 4 * (j + 1) + 4):
                        gens.append(v_proj_group(lt))
                    for et in range(NE):
                        gens.append(q_proj_group(j + 1, qtiles[j + 1], et))
                    n_fill_items += 9 * 12
                else:
                    # chunk 3 has no next-chunk projections: feed it ALL the
                    # output projection of chunks 0-2
                    for lt in range(0, 12):
                        for ec in range(2):
                            gens.append(out_proj_group(lt, ec))
                    n_fill_items += 5 * 24
                filler = chain(gens)
                nkt = 4 * (j + 1)
                steps = 8 * nkt
                QTc = qtiles[j]
                fill_acc = 0.0
                fill_done = 0
                for h in range(8):
                    hp = (h % 2) * 64
                    hb = h // 2
                    if h % 2 == 0:
                        att_sbs = [
                            attsbp.tile([128, 128], MM, tag=f"asb{qt}", name=f"asb{qt}")
                            for qt in range(4)
                        ]
                    PT = ptp.tile([128, NK, 512], MM, tag="pt", name="pt")
                    for kt in range(nkt):
                        s_ps = psS.tile([128, 512], F32, tag="s", name="s_ps")
                        nc.tensor.matmul(
                            s_ps[:],
                            KT[hp:hp + 64, hb, kt * 128:(kt + 1) * 128],
                            QTc[hp:hp + 64, hb, :],
                        )
                        nc.scalar.activation(PT[:, kt, :], s_ps[:], AF.Exp)
                        if kt >= nkt - 4:
                            nc.vector.tensor_mul(
                                PT[:, kt, :],
                                PT[:, kt, :],
                                masks_sb[:, kt - (nkt - 4), :],
                            )
                        fill_acc += n_fill_items / (steps * 1.2)
                        take = int(fill_acc) - fill_done
                        if take > 0:
                            drain(filler, take)
                            fill_done += take
                    for qt in range(4):
                        att_ps = psAV.tile([128, 65], F32, tag="av", name="att_ps")
                        nq = 4 * j + qt + 1
                        for kt in range(nq):
                            nc.tensor.matmul(
                                att_ps[:],
                                PT[:, kt, qt * 128:(qt + 1) * 128],
                                Vaug[:, kt, h, 0:65],
                                start=(kt == 0),
                                stop=(kt == nq - 1),
                            )
                        rc = smallp.tile([128, 1], F32, tag="rc", name="rc")
                        nc.vector.reciprocal(rc[:], att_ps[:, 64:65])
                        nc.vector.tensor_scalar_mul(
                            att_sbs[qt][:, hp:hp + 64], att_ps[:, 0:64], rc[:]
                        )
                        fill_acc += n_fill_items / (steps * 3.0)
                        take = int(fill_acc) - fill_done
                        if take > 0:
                            drain(filler, take)
                            fill_done += take
                    if h % 2 == 1:
                        for qt in range(4):
                            tpa = psX.tile([128, 128], F16, tag="x", name="tpa")
                            nc.tensor.transpose(tpa[:], att_sbs[qt][:], ident_h[:])
                            nc.vector.tensor_copy(
                                attT[:, hb, j * 512 + qt * 128:j * 512 + (qt + 1) * 128],
                                tpa[:],
                            )
                drain(filler, 10 ** 9)

            for lt in range(4 * (NJ - 1), 4 * (NJ - 1) + 4):
                for ec in range(2):
                    drain(out_proj_group(lt, ec), 99)

    nc.compile()
    return nc


def build_masks():
    kp = np.arange(128)[:, None]
    qf = np.arange(512)[None, :]
    return np.stack([(qf >= kp + 128 * m).astype(np.float16) for m in range(4)])


def _get_program():
    if "nc" not in _CACHE:
        _CACHE["nc"] = build_program()
    return _CACHE["nc"]


def make_in_maps(x, Wq, Wk, Wv, Wo):
    x = np.asarray(x, dtype=np.float32)
    Wq = np.asarray(Wq, dtype=np.float32)
    Wk = np.asarray(Wk, dtype=np.float32)
    Wv = np.asarray(Wv, dtype=np.float32)
    Wo = np.asarray(Wo, dtype=np.float32)
    masks = build_masks()
    in_maps = []
    for c in range(8):
        b, hg = c // 2, c % 2
        sl = slice(hg * E, (hg + 1) * E)
        in_maps.append(
            {
                "xb": np.ascontiguousarray(x[b]),
                "wq": np.ascontiguousarray(Wq[sl]),
                "wk": np.ascontiguousarray(Wk[sl]),
                "wv": np.ascontiguousarray(Wv[sl]),
                "wo": np.ascontiguousarray(Wo[:, sl]),
                "masks": masks,
            }
        )
    return in_maps


def kernel(x, Wq, Wk, Wv, Wo, **run_kwargs):
    from concourse import bass_utils

    nc = _get_program()
    in_maps = make_in_maps(x, Wq, Wk, Wv, Wo)
    res = bass_utils.run_bass_kernel_spmd(
        nc, in_maps, core_ids=list(range(8)), **run_kwargs
    )
    o = np.empty((B, L, D), np.float32)
    for b in range(B):
        o[b] = res.results[2 * b]["out"] + res.results[2 * b + 1]["out"]
    _CACHE["last_result"] = res
    return o
